# revision 7
# baseline (speedup 1.0000x reference)
# Causal self-attention (B=4, T=2048, C=768, H=12, D=64) on 8 TRN2 NeuronCores.
#
# Sharding: core = 2*b + g  (b = batch 0..3, g = head-group 0..1).
# Each core computes 6 heads (global heads 6g..6g+6) of batch b:
#   qkv projection (its weight columns), causal attention, and a partial
#   o_proj (its weight rows). Host sums the two partials per batch + b_o.
#
# Layout strategy (avoids all on-chip transposes of attention tensors):
#   xT   [C, T]   built via PE transpose of x tiles (the only transpose)
#   qT/kT [128, T] per head-PAIR (head A dims on partitions 0-63, B on 64-127)
#   S^T  [keys, queries] = kT.T @ qT  (contract D=64, row-packed pair)
#   P^T  = exp(S^T/8)  (ACT, scale folded into activation; no max-subtraction
#          needed: |S|/8 <= ~2 for these inputs)  + causal zero-mask (GPSIMD)
#   YT'  [65, q] = [V | 1].T @ P^T  (contract keys; row 64 = softmax denom l)
#   yt   = YT' / l  (DVE; l replicated across partitions via K=1 matmul)
#   y    = yt.T @ Wo_rows  (contract head dims)
#
# Matmul inputs are float32r (full-rate fp32 matmul); walrus requires the
# producing instruction to round to f32r, so those SBUF tiles are allocated
# as F32R and written by DVE/ACT (which round), with DMA-loaded weights
# staged through an f32 tile + cast copy.
import os
import sys
import numpy as np

_EXTRA_PATH = "/opt/trn_rl_repo"
if _EXTRA_PATH not in sys.path:
    sys.path.append(_EXTRA_PATH)

from contextlib import ExitStack

import concourse.bass as bass
import concourse.tile as tile
from concourse import bacc, mybir
from concourse.bass_utils import run_bass_kernel_spmd
from concourse.masks import make_identity

F32 = mybir.dt.float32
F32R = mybir.dt.float32r

B, T, C = 4, 2048, 768
H, D = 12, 64
H_LOC = 6            # heads per core
PAIRS = 3            # head pairs per core
CC = C // 128        # 6 contraction chunks over C
NT = T // 128        # 16 token chunks of 128
QB = T // 512        # 4 query blocks of 512
N_CORES = 8

Exp = mybir.ActivationFunctionType.Exp
IS_GE = mybir.AluOpType.is_ge


def _emit_kernel(tc, aps):
    nc = tc.nc
    x_ap = aps["x"]
    wq_ap, wk_ap, wv_ap = aps["wq"], aps["wk"], aps["wv"]
    bq_ap, bk_ap, bv_ap = aps["bq"], aps["bk"], aps["bv"]
    wo_ap = aps["wo"]
    y_ap = aps["y"]

    with ExitStack() as ctx:
        ctx.enter_context(nc.allow_low_precision(
            reason="float32r rounding on matmul inputs is intentional"))
        big = ctx.enter_context(tc.tile_pool(name="big", bufs=1))
        xt_pool = ctx.enter_context(tc.tile_pool(name="xtp", bufs=2))
        xin_pool = ctx.enter_context(tc.tile_pool(name="xin", bufs=2))
        p_pool = ctx.enter_context(tc.tile_pool(name="pp", bufs=3))
        y_pool = ctx.enter_context(tc.tile_pool(name="yp", bufs=2))
        st_pool = ctx.enter_context(tc.tile_pool(name="stp", bufs=2))
        sm_pool = ctx.enter_context(tc.tile_pool(name="smp", bufs=2))

        # ---- persistent SBUF tiles ----
        ident = big.tile([128, 128], F32, name="ident")
        make_identity(nc, ident)
        ones_sb = big.tile([1, 512], F32R, name="ones_sb")
        ones_stage = sm_pool.tile([1, 512], F32, name="ones_stage", tag="bst")
        nc.gpsimd.memset(ones_stage[:], 1.0)
        nc.vector.tensor_copy(ones_sb[:], ones_stage[:])

        # weights land in f32 staging tiles, then a DVE copy rounds to f32r
        wq_sb = big.tile([128, CC, 384], F32R, name="wq_sb")
        wk_sb = big.tile([128, CC, 384], F32R, name="wk_sb")
        wv_sb = big.tile([128, CC, 384], F32R, name="wv_sb")
        wo_sb = big.tile([128, PAIRS, 768], F32R, name="wo_sb")
        for w_ap, w_sb in ((wq_ap, wq_sb), (wk_ap, wk_sb), (wv_ap, wv_sb),
                           (wo_ap, wo_sb)):
            src = w_ap.rearrange("(o p) d -> p o d", p=128)
            oo, dd = src.shape[1], src.shape[2]
            o_per = max(1, 768 // dd)
            for c0 in range(0, oo, o_per):
                n_o = min(o_per, oo - c0)
                w_stage = xin_pool.tile([128, 768], F32, name="w_stage",
                                        tag="x_tile")
                stg = w_stage[:, :n_o * dd].rearrange("p (o d) -> p o d",
                                                      o=n_o)
                nc.sync.dma_start(stg, src[:, c0:c0 + n_o, :])
                nc.vector.tensor_copy(w_sb[:, c0:c0 + n_o, :], stg)
        bq_sb = big.tile([1, 384], F32R, name="bq_sb")
        bk_sb = big.tile([1, 384], F32R, name="bk_sb")
        bv_sb = big.tile([1, 384], F32R, name="bv_sb")
        for b_ap, b_sb in ((bq_ap, bq_sb), (bk_ap, bk_sb), (bv_ap, bv_sb)):
            b_stage = sm_pool.tile([1, 384], F32, name="b_stage", tag="bst")
            nc.sync.dma_start(b_stage[:], b_ap[:])
            nc.vector.tensor_copy(b_sb[:], b_stage[:])

        qT_sb = big.tile([128, PAIRS, T], F32R, name="qT_sb")
        kT_sb = big.tile([128, PAIRS, T], F32R, name="kT_sb")
        v_sb = big.tile([128, NT, H_LOC, 65], F32R, name="v_sb")
        vones_stage = xin_pool.tile([128, 96], F32, name="vones_stage",
                                    tag="x_tile")
        nc.gpsimd.memset(vones_stage[:], 1.0)
        nc.vector.tensor_copy(
            v_sb[:, :, :, 64],
            vones_stage.rearrange("p (a b) -> p a b", a=NT))
        yt_sb = big.tile([128, PAIRS, T], F32R, name="yt_sb")

        # ---- phase A: x load, transpose, qkv projection ----
        with tc.tile_pool(name="tr_ps", bufs=6, space="PSUM") as tr_psum, \
             tc.tile_pool(name="qkv_ps", bufs=2, space="PSUM") as qkv_psum:
            for tq in range(QB):  # 512-token blocks
                xT_blk = xt_pool.tile([128, CC, 512], F32R, name="xT_blk")
                trs = []
                for cc in range(CC):
                    ps_tr = tr_psum.tile([128, 512], F32, name="ps_tr",
                                         tag="ps_tr")
                    trs.append(ps_tr)
                for tsub in range(4):
                    x_tile = xin_pool.tile([128, 768], F32, name="x_tile")
                    t0 = (tq * 4 + tsub) * 128
                    nc.sync.dma_start(x_tile[:], x_ap[t0:t0 + 128, :])
                    for cc in range(CC):
                        nc.tensor.transpose(
                            trs[cc][:, tsub * 128:(tsub + 1) * 128],
                            x_tile[:, cc * 128:(cc + 1) * 128],
                            ident[:],
                        )
                for cc in range(CC):
                    nc.vector.tensor_copy(xT_blk[:, cc, :], trs[cc][:])

                qs = slice(tq * 512, (tq + 1) * 512)
                for pr in range(PAIRS):
                    prs = slice(pr * 128, (pr + 1) * 128)
                    for w_sb, b_sb, dst in (
                        (wq_sb, bq_sb, qT_sb),
                        (wk_sb, bk_sb, kT_sb),
                    ):
                        ps_qk = qkv_psum.tile([128, 512], F32, name="ps_qk",
                                              tag="qkv")
                        for cc in range(CC):
                            nc.tensor.matmul(
                                ps_qk[:],
                                w_sb[:, cc, prs],
                                xT_blk[:, cc, :],
                                start=(cc == 0), stop=False,
                            )
                        nc.tensor.matmul(  # + bias (rank-1 broadcast over t)
                            ps_qk[:], b_sb[0:1, prs], ones_sb[0:1, :],
                            start=False, stop=True,
                        )
                        nc.vector.tensor_copy(dst[:, pr, qs], ps_qk[:])
                for tsub in range(4):
                    tt = tq * 4 + tsub
                    ps_v = qkv_psum.tile([128, 384], F32, name="ps_v",
                                         tag="qkv")
                    for cc in range(CC):
                        nc.tensor.matmul(
                            ps_v[:],
                            xT_blk[:, cc, tsub * 128:(tsub + 1) * 128],
                            wv_sb[:, cc, :],
                            start=(cc == 0), stop=False,
                        )
                    nc.tensor.matmul(
                        ps_v[:], ones_sb[0:1, 0:128], bv_sb[0:1, :],
                        start=False, stop=True,
                    )
                    nc.vector.tensor_copy(
                        v_sb[:, tt, :, 0:64],
                        ps_v.rearrange("p (h d) -> p h d", h=H_LOC),
                    )

        # ---- phase B: attention ----
        with tc.tile_pool(name="s_ps", bufs=2, space="PSUM") as s_psum, \
             tc.tile_pool(name="yt_ps", bufs=2, space="PSUM") as yt_psum, \
             tc.tile_pool(name="rep_ps", bufs=2, space="PSUM") as rep_psum:
            for pr in range(PAIRS):
                hA, hB = 2 * pr, 2 * pr + 1
                for j in range(QB):
                    qs = slice(j * 512, (j + 1) * 512)
                    nkc = 4 * (j + 1)
                    ytA = yt_psum.tile([128, 512], F32, name="ytA", tag="yt")
                    ytB = yt_psum.tile([128, 512], F32, name="ytB", tag="yt")
                    for kc in range(nkc):
                        ks = slice(kc * 128, (kc + 1) * 128)
                        s_pair = s_psum.tile([128, 1024], F32, name="s_pair",
                                             tag="s")
                        # S^T = kT.T @ qT, K=64 row-packed: head A rows 0-63
                        # of the PE array, head B rows 64-127 (tile_position
                        # auto-derived from base partitions).
                        nc.tensor.matmul(
                            s_pair[:, 0:512],
                            kT_sb[0:64, pr, ks], qT_sb[0:64, pr, qs],
                            start=True, stop=True,
                        )
                        nc.tensor.matmul(
                            s_pair[:, 512:1024],
                            kT_sb[64:128, pr, ks],
                            qT_sb[64:128, pr, qs],
                            start=True, stop=True,
                        )
                        p_pair = p_pool.tile([128, 1024], F32R, name="p_pair",
                                             tag="p")
                        nc.scalar.activation(p_pair[:], s_pair[:], Exp,
                                             scale=0.125)
                        if kc >= 4 * j:
                            # diagonal chunk: zero P where query < key.
                            # keep iff (512j + f) - (128kc + p) >= 0, same
                            # predicate for both heads (dim of size 2, step 0)
                            nc.gpsimd.affine_select(
                                out=p_pair.rearrange("p (h q) -> p h q", h=2),
                                in_=p_pair.rearrange("p (h q) -> p h q", h=2),
                                compare_op=IS_GE,
                                fill=0.0,
                                base=512 * j - 128 * kc,
                                pattern=[[0, 2], [1, 512]],
                                channel_multiplier=-1,
                            )
                        nc.tensor.matmul(
                            ytA[0:65, :], v_sb[:, kc, hA, :],
                            p_pair[:, 0:512],
                            start=(kc == 0), stop=(kc == nkc - 1),
                        )
                        nc.tensor.matmul(
                            ytB[0:65, :], v_sb[:, kc, hB, :],
                            p_pair[:, 512:1024],
                            start=(kc == 0), stop=(kc == nkc - 1),
                        )
                    # normalize: yt = YT'[0:64] * (1/l), l = YT'[64]
                    linvA = sm_pool.tile([1, 512], F32R, name="linvA",
                                         tag="linv")
                    nc.vector.reciprocal(linvA[:], ytA[64:65, :])
                    repA = rep_psum.tile([64, 512], F32, name="repA",
                                         tag="rep")
                    nc.tensor.matmul(repA[:], ones_sb[0:1, 0:64],
                                     linvA[:], start=True, stop=True)
                    repA_sb = st_pool.tile([64, 512], F32, name="repA_sb",
                                           tag="rep_sb")
                    nc.vector.tensor_copy(repA_sb[:], repA[:])
                    nc.vector.tensor_mul(yt_sb[0:64, pr, qs],
                                         ytA[0:64, :], repA_sb[:])

                    linvB = sm_pool.tile([1, 512], F32R, name="linvB",
                                         tag="linv")
                    nc.vector.reciprocal(linvB[:], ytB[64:65, :])
                    repB = rep_psum.tile([64, 512], F32, name="repB",
                                         tag="rep")
                    nc.tensor.matmul(repB[:], ones_sb[0:1, 0:64],
                                     linvB[:], start=True, stop=True)
                    repB_sb = st_pool.tile([64, 512], F32, name="repB_sb",
                                           tag="rep_sb")
                    nc.vector.tensor_copy(repB_sb[:], repB[:])
                    stB = st_pool.tile([64, 512], F32R, name="stB")
                    nc.vector.tensor_mul(stB[:], ytB[0:64, :], repB_sb[:])
                    # head B lives on partitions 64-127 of yt_sb: DMA does the
                    # partition shift (DVE lanes cannot).
                    nc.sync.dma_start(yt_sb[64:128, pr, qs], stB[:])

        # ---- phase C: o_proj partial:  y = yt.T @ wo_rows ----
        with tc.tile_pool(name="o_ps", bufs=2, space="PSUM") as o_psum:
            for tt in range(NT):
                ts_ = slice(tt * 128, (tt + 1) * 128)
                ps1 = o_psum.tile([128, 512], F32, name="ps_o1", tag="o1")
                ps2 = o_psum.tile([128, 256], F32, name="ps_o2", tag="o2")
                for pr in range(PAIRS):
                    nc.tensor.matmul(ps1[:], yt_sb[:, pr, ts_],
                                     wo_sb[:, pr, 0:512],
                                     start=(pr == 0), stop=(pr == PAIRS - 1))
                for pr in range(PAIRS):
                    nc.tensor.matmul(ps2[:], yt_sb[:, pr, ts_],
                                     wo_sb[:, pr, 512:768],
                                     start=(pr == 0), stop=(pr == PAIRS - 1))
                y_sb = y_pool.tile([128, 768], F32, name="y_sb")
                nc.vector.tensor_copy(y_sb[:, 0:512], ps1[:])
                nc.vector.tensor_copy(y_sb[:, 512:768], ps2[:])
                nc.sync.dma_start(y_ap[ts_, :], y_sb[:])


_COMPILED = None


def _build():
    global _COMPILED
    if _COMPILED is not None:
        return _COMPILED
    nc = bacc.Bacc("TRN2", target_bir_lowering=False, debug=False)
    aps = {
        "x": nc.dram_tensor("x", [T, C], F32, kind="ExternalInput").ap(),
        "wq": nc.dram_tensor("wq", [C, 384], F32, kind="ExternalInput").ap(),
        "wk": nc.dram_tensor("wk", [C, 384], F32, kind="ExternalInput").ap(),
        "wv": nc.dram_tensor("wv", [C, 384], F32, kind="ExternalInput").ap(),
        "bq": nc.dram_tensor("bq", [1, 384], F32, kind="ExternalInput").ap(),
        "bk": nc.dram_tensor("bk", [1, 384], F32, kind="ExternalInput").ap(),
        "bv": nc.dram_tensor("bv", [1, 384], F32, kind="ExternalInput").ap(),
        "wo": nc.dram_tensor("wo", [384, C], F32, kind="ExternalInput").ap(),
        "y": nc.dram_tensor("y", [T, C], F32, kind="ExternalOutput").ap(),
    }
    with tile.TileContext(nc) as tc:
        _emit_kernel(tc, aps)
    nc.compile()
    _COMPILED = nc
    return nc


last_results = None


def kernel(x, W_attn, b_attn, W_o, b_o):
    global last_results
    x = np.asarray(x, dtype=np.float32)
    W_attn = np.asarray(W_attn, dtype=np.float32)
    b_attn = np.asarray(b_attn, dtype=np.float32)
    W_o = np.asarray(W_o, dtype=np.float32)
    b_o = np.asarray(b_o, dtype=np.float32)

    nc = _build()

    in_maps = []
    for core in range(N_CORES):
        b, g = core // 2, core % 2
        cols = slice(g * 384, (g + 1) * 384)
        in_maps.append({
            "x": np.ascontiguousarray(x[b]),
            "wq": np.ascontiguousarray(W_attn[:, cols]),
            "wk": np.ascontiguousarray(W_attn[:, 768:][:, cols]),
            "wv": np.ascontiguousarray(W_attn[:, 1536:][:, cols]),
            "bq": np.ascontiguousarray(b_attn[None, cols]),
            "bk": np.ascontiguousarray(b_attn[None, 768:][:, cols]),
            "bv": np.ascontiguousarray(b_attn[None, 1536:][:, cols]),
            "wo": np.ascontiguousarray(W_o[g * 384:(g + 1) * 384, :]),
        })

    res = run_bass_kernel_spmd(nc, in_maps, core_ids=list(range(N_CORES)))
    last_results = res

    y = np.empty((B, T, C), dtype=np.float32)
    for b in range(B):
        y[b] = res.results[2 * b]["y"] + res.results[2 * b + 1]["y"] + b_o
    return y
